# revision 50
# baseline (speedup 1.0000x reference)
"""Multi-head attention (B=4, S=2048, D=1024, H=16) on 8 Trainium2 cores.

Sharding: (batch, head-group) grid — core c handles batch c//2, heads
(c%2)*8..(c%2)*8+8. Zero duplicated FLOPs; host sums the two partial
out-projections per batch and adds bo.

v9 design, ~397us (from trace analysis of the 443us v2: PE busy 358us/80%,
Scalar 202us exp, DVE 168us, 90us PE idle in startup/stalls/tail):
  * fp8 was tried (v3/v4) and is numerically DEAD for the 2e-2 gate:
    high-variance score rows concentrate softmax mass (k_eff ~ 40), so
    the ~4-7% fp8 quantization of P (or of q/k/v via fp8 projections)
    lands at 3-6e-2 output error (verified on HW and in numpy).
    Everything stays fp16; the late phase is PE-bound at ~95% occupancy.
  * Software-pipelined attention emission: AV(ktp-1) is emitted after
    scores(ktp), so the in-order PE queue streams scores while the exp
    engines produce P for the previous pair (v5 measured ~1us PE stalls
    per pair without this).
  * Per k-tile-pair, j=0's exp runs on ScalarE (exact ACTIVATE) and
    j=1's on VectorE (fp16 Schraudolph via uint16 convert; negatives
    saturate to +0.0), halving the exp latency per pair; SS_KTPS shifts
    two late-phase pairs fully onto ScalarE to balance VectorE's chores.
  * ~72 junk warm-up matmuls at t=0 bridge the input-DMA wait (~29us) and open
    the HAM clock gate (1.2 -> 2.4 GHz) before the first real matmul.
  * Host DRAM layouts are DMA-native (8KB descriptors: weights [p,c,n],
    x windows [qs,p,c*n]); weight loads, out-writes, and alternate startup
    K-windows ride the
    Activation HWDGE queue in parallel with x-windows/normalize chains
    on the Sync queue.
  * Output staged and written as fp16 (halves the 8.4MB/core out write;
    the host sums the two partial projections in fp32).
  * Kernel tail: three deferred out-projection tiles fill the last
    softmax-normalize gap; the final reciprocal broadcast skips the DMA
    round trip via a K=1 PE outer product (ones x recip-row).
  * UNFINISHED idea (~2us EV): the tail's two [1,512] reciprocals
    serialize on DVE at 3.3us each (recip is free-dim-serial per lane);
    staging both denominator rows into one [2,512] tile and running ONE
    reciprocal would halve that. Blocked on a bass matmul assert
    (lhsT/rhs base_partition match for the K=1 outer product) that
    needs explicit tile_position or partition-0-aligned staging.
  * Tried and REGRESSED (do not revisit blindly): splitting AON into
    per-slice tensors to break a false dep (433us -- scheduler side
    effects), batching the two per-unit reciprocal chains (467us), and
    a 44-warmup + xpool=6 combo (467us). The tile scheduler is very
    sensitive to emission/dependency perturbations; change one knob at
    a time and re-measure.
"""

import numpy as np
import ml_dtypes

import bass_rust
import concourse.bass as bass
import concourse.tile as tile
from concourse import mybir

F32 = mybir.dt.float32
U16 = mybir.dt.uint16
MMD = mybir.dt.float16     # fp16 matmul operand dtype

B, S, D = 4, 2048, 1024
NH, DK = 16, 64            # total heads, head dim
HG = 8                     # heads per core (head group)
DHG = HG * DK              # 512 features per head group
NP = 4                     # pairs of heads per core
QS = 512                   # q-slice size
NQS = S // QS              # 4
KT = S // 128              # 16 k-tiles
KTP = KT // 2              # 8 k-tile pairs
CT = D // 128              # 8 contraction chunks for projections
VW = DK + 1                # 65: V columns per head incl. ones column

# fp16 Schraudolph fast-exp: round(s*1024*0.125/ln2 + 15360-57.77) as
# uint16 IS the fp16 bit pattern of exp(s/8) to within +-3.5% (zero mean)
FE_A = 1024.0 * 0.125 / float(np.log(2.0))
FE_B = 15360.0 - 57.77

# k-tile-pairs where ScalarE takes BOTH exp tiles (instead of the default
# j0->Scalar, j1->Vector split): late q-slices shift work off VectorE,
# which also carries the reciprocal/copy chores.
SS_KTPS = {0: (), 1: (), 2: (2, 5), 3: (2, 5)}


def split_multi_waits(nc):
    """This toolchain's walrus accepts only ONE sync-wait per instruction;
    Tile attaches several (one per producer proc). Hoist all but one wait
    onto single-wait NOPs inserted just before the instruction on the same
    engine (engines are in-order, so semantics are identical)."""
    uid = 0
    for f in nc.m.functions:
        for bb in f.blocks:
            il = bb.instructions
            i = 0
            while i < len(il):
                inst = il[i]
                si = inst.sync_info
                if si is not None and len(si.on_wait) > 1:
                    waits = list(si.on_wait)
                    inst.sync_info = bass_rust.SyncInfo(
                        on_wait=[waits[-1]], on_update=list(si.on_update)
                    )
                    for w in waits[:-1]:
                        nop = mybir.InstNoOp(
                            name=f"WSPLIT-{uid}",
                            engine=inst.engine,
                            bass_nofuse=True,
                            sync_info=bass_rust.SyncInfo(
                                on_wait=[w], on_update=[]
                            ),
                        )
                        uid += 1
                        il.insert(i, nop)
                        i += 1
                i += 1


def bcast_ap(ap, parts, n):
    """Partition-broadcast view of a DRAM row AP: [[0,parts],[1,n]]."""
    return bass.AP(tensor=ap.tensor, offset=ap.offset, ap=[[0, parts], [1, n]])


def build_kernel():
    nc = bass.Bass(trn_type="TRN2")

    # host layouts are already DMA-native (see _prep_inputs)
    xq = nc.dram_tensor("xq", (NQS, 128, CT * QS), MMD, kind="ExternalInput")
    xk = nc.dram_tensor("xk", (NQS, 128, CT * QS), MMD, kind="ExternalInput")
    xv = nc.dram_tensor("xv", (NQS, 128, CT * QS), MMD, kind="ExternalInput")
    wq = nc.dram_tensor("wq", (128, CT, DHG), MMD, kind="ExternalInput")
    wk = nc.dram_tensor("wk", (128, CT, DHG), MMD, kind="ExternalInput")
    wv = nc.dram_tensor("wv", (128, CT, DHG), MMD, kind="ExternalInput")
    wo = nc.dram_tensor("wo", (128, NP, D), MMD, kind="ExternalInput")
    bq = nc.dram_tensor("bq", (128, NP), F32, kind="ExternalInput")
    bk = nc.dram_tensor("bk", (128, NP), F32, kind="ExternalInput")
    bv = nc.dram_tensor("bv", (DHG,), F32, kind="ExternalInput")
    out = nc.dram_tensor("out", (S, D), MMD, kind="ExternalOutput")

    from contextlib import ExitStack

    with tile.TileContext(nc) as tc, ExitStack() as ctx:
        persist = ctx.enter_context(tc.tile_pool(name="persist", bufs=1))
        KT_sb = persist.tile([128, NP, S], MMD)        # K^T: pair p rows
        QT_sb = persist.tile([128, NP, S], MMD)        # Q^T
        V_sb = persist.tile([128, KT, HG, VW], MMD)    # V token-major + ones
        AON = persist.tile([128, NP, S], MMD)          # normalized AO^T
        wk_sb = persist.tile([128, CT, DHG], MMD)
        wq_sb = persist.tile([128, CT, DHG], MMD)
        wv_sb = persist.tile([128, CT, DHG], MMD)
        wo_sb = persist.tile([128, NP, D], MMD)
        bq_sb = persist.tile([128, NP], F32)
        bk_sb = persist.tile([128, NP], F32)
        bv_bc = persist.tile([128, DHG], F32)

        # wk first so the first K-projection matmuls start early
        nc.sync.dma_start(wk_sb[:], wk[:])
        nc.vector.memset(V_sb[:, :, :, DK], 1.0)       # ones columns
        # HAM warm-up: ~20 junk matmuls keep the PE busy from ~t=1us so
        # the clock gate opens (1.2 -> 2.4 GHz) before the first real
        # projection matmul, and the input-DMA wait isn't wasted
        wdum = persist.tile([128, QS], MMD)
        nc.vector.memset(wdum[:], 0.0)
        ones1 = persist.tile([1, DK], F32)             # tail rb broadcast
        nc.vector.memset(ones1[:], 1.0)
        ones1 = persist.tile([1, DK], F32)             # tail rb broadcast
        nc.vector.memset(ones1[:], 1.0)

        pmm = ctx.enter_context(tc.tile_pool(name="pmm", bufs=1, space="PSUM"))
        xpool = ctx.enter_context(tc.tile_pool(name="xw", bufs=7))
        vxpool = ctx.enter_context(tc.tile_pool(name="vxw", bufs=2))
        ptp = ctx.enter_context(tc.tile_pool(name="ptile", bufs=5))
        npool = ctx.enter_context(tc.tile_pool(name="norm", bufs=3))
        opool = ctx.enter_context(tc.tile_pool(name="ostage", bufs=3))
        dpool = ctx.enter_context(
            tc.tile_pool(name="dscratch", bufs=3, space="DRAM")
        )

        def window(xdram, qs, pool, dt, q=None):
            """One contiguous DMA for a 512-token slice (all 8 chunks)."""
            xc = pool.tile([128, CT, QS], dt, tag="xw", name="xw")
            (q or nc.sync).dma_start(
                xc[:], xdram[qs].rearrange("p (c n) -> p c n", n=QS)
            )
            return xc

        def kq_jt(win, w_sb, dst, b_sb, jt, qs):
            """dst[:, jt, qs] = w[:, :, jt].T @ x^T[:, qs] + bias."""
            ps = pmm.tile([128, QS], F32, tag="pj", name="pj", bufs=2)
            for ct in range(CT):
                nc.tensor.matmul(
                    ps[:],
                    w_sb[:, ct, jt * 128:(jt + 1) * 128],
                    win[:, ct, :],
                    start=(ct == 0), stop=(ct == CT - 1),
                )
            nc.vector.tensor_scalar_add(
                dst[:, jt, qs * QS:(qs + 1) * QS], ps[:], b_sb[:, jt:jt + 1]
            )

        def v_tiles(qs):
            """V_sb tok-tiles for one 512-token slice (4 tiles)."""
            win = window(xv, qs, vxpool, MMD)
            for i in range(4):
                tt = qs * 4 + i
                ps = pmm.tile([128, DHG], F32, tag="pj", name="pjv", bufs=2)
                for ct in range(CT):
                    nc.tensor.matmul(
                        ps[:],
                        win[:, ct, i * 128:(i + 1) * 128],
                        wv_sb[:, ct, :],
                        start=(ct == 0), stop=(ct == CT - 1),
                    )
                nc.vector.tensor_add(
                    V_sb[:, tt, :, 0:DK],
                    ps[:].rearrange("p (h d) -> p h d", d=DK),
                    bv_bc[:].rearrange("p (h d) -> p h d", d=DK),
                )

        def attention(p, qsb):
            """One head-pair over one 512-wide q-slice.

            Scores run k-major in the PE's 64x128 row-tiled mode (two heads
            at base partitions 0/64 execute concurrently). Softmax weights
            land as P^T; a ones column in V accumulates the denominator in
            PSUM row 64 during the PV matmuls. ScalarE k-tile-pairs emit P
            as fp8e4 and contract two k-tiles per DoubleRow AV pass;
            VectorE pairs keep fp16 P and classic AV (V stays fp8 lhsT).
            """
            q0 = qsb * QS
            ss_ktps = SS_KTPS[qsb]
            ao = [
                pmm.tile([VW, QS], F32, tag=f"ao{h2}", name=f"ao{h2}")
                for h2 in range(2)
            ]

            def av(pt, ktp):
                for j in range(2):
                    kt = 2 * ktp + j
                    for h2 in range(2):
                        nc.tensor.matmul(
                            ao[h2][:],
                            V_sb[:, kt, 2 * p + h2, 0:VW],
                            pt[j][:, h2, :],
                            start=(kt == 0), stop=(kt == KT - 1),
                        )

            # software-pipelined emission: AV(ktp-1) is emitted AFTER
            # scores(ktp), so the in-order PE queue streams scores while
            # the exp engines produce P for the previous pair -- the exp
            # latency hides behind score matmuls instead of stalling AV.
            prev = None
            for ktp in range(KTP):
                st = [
                    pmm.tile([128, 2, QS], F32, tag="st", name=f"st{j}",
                             bufs=2)
                    for j in range(2)
                ]
                for j in range(2):
                    kt = 2 * ktp + j
                    for h2 in range(2):
                        lo, hi = h2 * DK, h2 * DK + DK
                        nc.tensor.matmul(
                            st[j][:, h2, :],
                            KT_sb[lo:hi, p, kt * 128:(kt + 1) * 128],
                            QT_sb[lo:hi, p, q0:q0 + QS],
                            start=True, stop=True,
                        )
                pt = [
                    ptp.tile([128, 2, QS], MMD, tag="pt", name=f"pt{j}")
                    for j in range(2)
                ]
                for j in range(2):
                    if j == 1 and ktp not in ss_ktps:
                        nc.vector.tensor_scalar(
                            pt[j][:].bitcast(U16),
                            st[j][:],
                            FE_A, FE_B,
                            mybir.AluOpType.mult, mybir.AluOpType.add,
                        )
                    else:
                        nc.scalar.activation(
                            pt[j][:], st[j][:],
                            mybir.ActivationFunctionType.Exp,
                            scale=0.125,
                        )
                if prev is not None:
                    av(*prev)
                prev = (pt, ktp)
            av(*prev)
            tail = qsb == NQS - 1 and p == NP - 1
            for h2 in range(2):
                # copy to SBUF promptly so PSUM frees fast
                aos = npool.tile([VW, QS], F32, tag="aos", name="aos")
                if h2 == 0:
                    nc.scalar.copy(aos[:], ao[h2][:])
                else:
                    nc.vector.tensor_copy(aos[:], ao[h2][:])
                if tail:
                    # kernel-tail short chain: 1-lane reciprocal, then the
                    # PE broadcasts it across partitions with a K=1 outer
                    # product (ones x recip-row) -- no DMA round trip
                    # through the congested Sync queue
                    rr = npool.tile([1, QS], F32, tag="rr", name="rr")
                    nc.vector.reciprocal(rr[:], ao[h2][DK:VW, :])
                    rbp = pmm.tile([DK, QS], F32, tag=f"ao{h2}",
                                   name="rbp")
                    nc.tensor.matmul(
                        rbp[:], ones1[0:1, :], rr[:],
                        start=True, stop=True,
                    )
                    nc.vector.tensor_mul(
                        AON[h2 * DK:(h2 + 1) * DK, p, q0:q0 + QS],
                        aos[0:DK, :],
                        rbp[:],
                    )
                    continue
                # full-lane recip via [1,512] -> [128,4] DRAM reshape
                rcd = dpool.tile([1, QS], F32, tag="rcd", name="rcd")
                dn = dpool.tile([1, QS], F32, tag="dn", name="dn")
                nc.sync.dma_start(dn[:], aos[DK:VW, :])
                rc = npool.tile([128, 4], F32, tag="rc", name="rc")
                nc.sync.dma_start(
                    rc[:], dn[:].rearrange("x (p j) -> (x p) j", j=4)
                )
                nc.vector.reciprocal(rc[:], rc[:])
                nc.sync.dma_start(
                    rcd[:].rearrange("x (p j) -> (x p) j", j=4), rc[:]
                )
                rb = npool.tile([DK, QS], F32, tag="rb", name="rb")
                nc.sync.dma_start(rb[:], bcast_ap(rcd[:], DK, QS))
                # gpsimd is idle so it takes the normalize muls
                nc.gpsimd.tensor_mul(
                    AON[h2 * DK:(h2 + 1) * DK, p, q0:q0 + QS],
                    aos[0:DK, :],
                    rb[:],
                )

        def outproj_tile(qsb, tt, last=False):
            """Out-projection for token tile tt (128 rows) of q-slice qsb."""
            q0 = qsb * QS
            ot = opool.tile([128, D], MMD, tag="ot", name="ot")
            for oh in range(2):
                po = pmm.tile([128, 512], F32, tag="pj", name="po", bufs=2)
                for ci in range(NP):
                    nc.tensor.matmul(
                        po[:],
                        AON[:, ci, q0 + tt * 128:q0 + (tt + 1) * 128],
                        wo_sb[:, ci, oh * 512:(oh + 1) * 512],
                        start=(ci == 0), stop=(ci == NP - 1),
                    )
                # VectorE carries the fast-exp + chores; PSUM evacuation
                # goes to ScalarE which has slack -- except at the very
                # end, where both engines split the two halves
                if last and oh == 0:
                    nc.vector.tensor_copy(
                        ot[:, oh * 512:(oh + 1) * 512], po[:])
                else:
                    nc.scalar.copy(ot[:, oh * 512:(oh + 1) * 512], po[:])
            # out-writes ride the Activation HWDGE queue: the Sync queue
            # carries windows + normalize chains and backs up
            nc.scalar.dma_start(
                out[q0 + tt * 128:q0 + (tt + 1) * 128, :], ot[:])

        # ---- emission schedule ---------------------------------------------
        # Normal priority: K proj (all pairs, windows shared across pairs),
        # Q pair-0 slice-0, V, then the attention stream + out-projections.
        # Remaining Q projections are demoted to background priority: the
        # scheduler pulls them early only when a data dependency demands it,
        # and otherwise uses them to fill PE idle slots.
        for w in range(72):
            pw = pmm.tile([128, QS], F32, tag="pj", name="pjw", bufs=2)
            nc.tensor.matmul(pw[:], wdum[:, 0:128], wdum[:],
                             start=True, stop=True)
        kwins = [window(xk, qs, xpool, MMD,
                        q=(nc.scalar if qs >= 2 else nc.sync))
                 for qs in range(NQS)]
        qwin0 = window(xq, 0, xpool, MMD)
        nc.sync.dma_start(bq_sb[:], bq[:])
        nc.sync.dma_start(bk_sb[:], bk[:])
        # weight loads ride the Activation HWDGE queue so they don't
        # queue behind the 1MB x-window transfers on the Sync queue
        nc.scalar.dma_start(wq_sb[:], wq[:])
        nc.scalar.dma_start(wv_sb[:], wv[:])
        nc.scalar.dma_start(bv_bc[:], bcast_ap(bv[:], 128, DHG))
        nc.scalar.dma_start(wo_sb[:], wo[:])
        for qs in range(NQS):
            kq_jt(kwins[qs], wk_sb, KT_sb, bk_sb, 0, qs)
        kq_jt(qwin0, wq_sb, QT_sb, bq_sb, 0, 0)
        for qs in range(NQS):
            v_tiles(qs)
        for jt in range(1, NP):
            for qs in range(NQS):
                kq_jt(kwins[qs], wk_sb, KT_sb, bk_sb, jt, qs)

        with tc.high_priority(offset=-(10 ** 6)):
            for jt in range(1, NP):
                kq_jt(qwin0, wq_sb, QT_sb, bq_sb, jt, 0)
            for qs in range(1, NQS):
                qwin = window(xq, qs, xpool, MMD)
                for jt in range(NP):
                    kq_jt(qwin, wq_sb, QT_sb, bq_sb, jt, qs)

        # two of each slice's out-projection tiles are deferred to the very
        # end: they fill the PE while the last softmax-normalize chain runs
        # (and keep the HAM clock warm for the final out-projections)
        for qsb in range(NQS):
            for p in range(NP):
                attention(p, qsb)
                if qsb > 0 and (p < 1 or qsb < NQS - 1):
                    outproj_tile(qsb - 1, p)
        for tt in range(1, NQS):
            outproj_tile(NQS - 2, tt)
        for tt in range(NQS):
            outproj_tile(NQS - 1, tt, last=(tt == NQS - 1))

    split_multi_waits(nc)
    return nc


def _prep_inputs(query, key, value, Wq, bq, Wk, bk, Wv, bv, Wo, bo):
    """Build the 8 per-core input maps (DMA-native DRAM layouts)."""
    F8NP = ml_dtypes.float8_e4m3

    def c16(a):
        return np.ascontiguousarray(a.astype(np.float16))

    def c8(a):
        return np.ascontiguousarray(
            np.clip(a, -240.0, 240.0).astype(F8NP))

    def xprep(x, b, dt):
        # x[b].T is (D, S) = (c*128, qs*512 tokens) feature-major ->
        # [qs, p, c*512] so a window DMA is contiguous per partition
        a = x[b].T.reshape(CT, 128, NQS, QS).transpose(2, 1, 0, 3)
        a = a.reshape(NQS, 128, CT * QS)
        return c16(a) if dt == "f16" else c8(a)

    def wprep(Wt, dt):
        # W[rows,:].T is (D, DHG) = (c*128, n) -> [p, c, n]
        a = Wt.reshape(CT, 128, DHG).transpose(1, 0, 2)
        return c16(a) if dt == "f16" else c8(a)

    in_maps = []
    for c in range(8):
        b, g = divmod(c, 2)
        rows = slice(g * DHG, (g + 1) * DHG)
        wo_a = Wo[:, rows].T.reshape(NP, 128, D).transpose(1, 0, 2)
        in_maps.append({
            "xq": xprep(query, b, "f16"),
            "xk": xprep(key, b, "f16"),
            "xv": xprep(value, b, "f16"),
            "wq": wprep(Wq[rows, :].T, "f16"),
            "wk": wprep(Wk[rows, :].T, "f16"),
            "wv": wprep(Wv[rows, :].T, "f16"),
            "wo": c16(wo_a),
            "bq": np.ascontiguousarray(bq[rows].reshape(NP, 128).T),
            "bk": np.ascontiguousarray(bk[rows].reshape(NP, 128).T),
            "bv": np.ascontiguousarray(bv[rows]),
        })
    return in_maps


_NC_CACHE = None


def run(inputs, trace=False):
    """Returns (full_output, BassKernelResults)."""
    global _NC_CACHE
    from concourse.bass_utils import run_bass_kernel_spmd

    inputs = {k: np.asarray(v, np.float32) for k, v in inputs.items()}
    in_maps = _prep_inputs(**inputs)
    if _NC_CACHE is None:
        _NC_CACHE = build_kernel()
    res = run_bass_kernel_spmd(
        _NC_CACHE, in_maps, core_ids=list(range(8)), trace=trace
    )
    bo = inputs["bo"]
    full = np.empty((B, S, D), np.float32)
    for b in range(B):
        full[b] = (res.results[2 * b]["out"].astype(np.float32)
                   + res.results[2 * b + 1]["out"].astype(np.float32) + bo)
    return full, res


def kernel(**inputs):
    return run(inputs, trace=False)[0]


# revision 52
# speedup vs baseline: 1.0027x; 1.0027x over previous
"""Multi-head attention (B=4, S=2048, D=1024, H=16) on 8 Trainium2 cores.

Sharding: (batch, head-group) grid — core c handles batch c//2, heads
(c%2)*8..(c%2)*8+8. Zero duplicated FLOPs; host sums the two partial
out-projections per batch and adds bo.

v9 design, ~397us (from trace analysis of the 443us v2: PE busy 358us/80%,
Scalar 202us exp, DVE 168us, 90us PE idle in startup/stalls/tail):
  * fp8 was tried (v3/v4) and is numerically DEAD for the 2e-2 gate:
    high-variance score rows concentrate softmax mass (k_eff ~ 40), so
    the ~4-7% fp8 quantization of P (or of q/k/v via fp8 projections)
    lands at 3-6e-2 output error (verified on HW and in numpy).
    Everything stays fp16; the late phase is PE-bound at ~95% occupancy.
  * Software-pipelined attention emission: AV(ktp-1) is emitted after
    scores(ktp), so the in-order PE queue streams scores while the exp
    engines produce P for the previous pair (v5 measured ~1us PE stalls
    per pair without this).
  * Per k-tile-pair, j=0's exp runs on ScalarE (exact ACTIVATE) and
    j=1's on VectorE (fp16 Schraudolph via uint16 convert; negatives
    saturate to +0.0), halving the exp latency per pair; SS_KTPS shifts
    two late-phase pairs fully onto ScalarE to balance VectorE's chores.
  * ~72 junk warm-up matmuls at t=0 bridge the input-DMA wait (~29us) and open
    the HAM clock gate (1.2 -> 2.4 GHz) before the first real matmul.
  * Host DRAM layouts are DMA-native (8KB descriptors: weights [p,c,n],
    x windows [qs,p,c*n]); weight loads, out-writes, and alternate startup
    K-windows ride the
    Activation HWDGE queue in parallel with x-windows/normalize chains
    on the Sync queue.
  * Output staged and written as fp16 (halves the 8.4MB/core out write;
    the host sums the two partial projections in fp32).
  * Kernel tail: three deferred out-projection tiles fill the last
    softmax-normalize gap; the final reciprocal broadcast skips the DMA
    round trip via a K=1 PE outer product (ones x recip-row).
  * Tail denominators staged at partitions 0 and 32 of one tile so a
    SINGLE reciprocal covers both heads (DVE recip is free-dim-serial
    per lane, 3.3us per call; partition 32 is row-group aligned so the
    K=1 outer products auto-derive a valid tile_position). -2.1us mean.
  * Tried and REGRESSED (do not revisit blindly): splitting AON into
    per-slice tensors to break a false dep (433us -- scheduler side
    effects), batching the two per-unit reciprocal chains (467us), and
    a 44-warmup + xpool=6 combo (467us). The tile scheduler is very
    sensitive to emission/dependency perturbations; change one knob at
    a time and re-measure.
"""

import numpy as np
import ml_dtypes

import bass_rust
import concourse.bass as bass
import concourse.tile as tile
from concourse import mybir

F32 = mybir.dt.float32
U16 = mybir.dt.uint16
MMD = mybir.dt.float16     # fp16 matmul operand dtype

B, S, D = 4, 2048, 1024
NH, DK = 16, 64            # total heads, head dim
HG = 8                     # heads per core (head group)
DHG = HG * DK              # 512 features per head group
NP = 4                     # pairs of heads per core
QS = 512                   # q-slice size
NQS = S // QS              # 4
KT = S // 128              # 16 k-tiles
KTP = KT // 2              # 8 k-tile pairs
CT = D // 128              # 8 contraction chunks for projections
VW = DK + 1                # 65: V columns per head incl. ones column

# fp16 Schraudolph fast-exp: round(s*1024*0.125/ln2 + 15360-57.77) as
# uint16 IS the fp16 bit pattern of exp(s/8) to within +-3.5% (zero mean)
FE_A = 1024.0 * 0.125 / float(np.log(2.0))
FE_B = 15360.0 - 57.77

# k-tile-pairs where ScalarE takes BOTH exp tiles (instead of the default
# j0->Scalar, j1->Vector split): late q-slices shift work off VectorE,
# which also carries the reciprocal/copy chores.
SS_KTPS = {0: (), 1: (), 2: (2, 5), 3: (2, 5)}


def split_multi_waits(nc):
    """This toolchain's walrus accepts only ONE sync-wait per instruction;
    Tile attaches several (one per producer proc). Hoist all but one wait
    onto single-wait NOPs inserted just before the instruction on the same
    engine (engines are in-order, so semantics are identical)."""
    uid = 0
    for f in nc.m.functions:
        for bb in f.blocks:
            il = bb.instructions
            i = 0
            while i < len(il):
                inst = il[i]
                si = inst.sync_info
                if si is not None and len(si.on_wait) > 1:
                    waits = list(si.on_wait)
                    inst.sync_info = bass_rust.SyncInfo(
                        on_wait=[waits[-1]], on_update=list(si.on_update)
                    )
                    for w in waits[:-1]:
                        nop = mybir.InstNoOp(
                            name=f"WSPLIT-{uid}",
                            engine=inst.engine,
                            bass_nofuse=True,
                            sync_info=bass_rust.SyncInfo(
                                on_wait=[w], on_update=[]
                            ),
                        )
                        uid += 1
                        il.insert(i, nop)
                        i += 1
                i += 1


def bcast_ap(ap, parts, n):
    """Partition-broadcast view of a DRAM row AP: [[0,parts],[1,n]]."""
    return bass.AP(tensor=ap.tensor, offset=ap.offset, ap=[[0, parts], [1, n]])


def build_kernel():
    nc = bass.Bass(trn_type="TRN2")

    # host layouts are already DMA-native (see _prep_inputs)
    xq = nc.dram_tensor("xq", (NQS, 128, CT * QS), MMD, kind="ExternalInput")
    xk = nc.dram_tensor("xk", (NQS, 128, CT * QS), MMD, kind="ExternalInput")
    xv = nc.dram_tensor("xv", (NQS, 128, CT * QS), MMD, kind="ExternalInput")
    wq = nc.dram_tensor("wq", (128, CT, DHG), MMD, kind="ExternalInput")
    wk = nc.dram_tensor("wk", (128, CT, DHG), MMD, kind="ExternalInput")
    wv = nc.dram_tensor("wv", (128, CT, DHG), MMD, kind="ExternalInput")
    wo = nc.dram_tensor("wo", (128, NP, D), MMD, kind="ExternalInput")
    bq = nc.dram_tensor("bq", (128, NP), F32, kind="ExternalInput")
    bk = nc.dram_tensor("bk", (128, NP), F32, kind="ExternalInput")
    bv = nc.dram_tensor("bv", (DHG,), F32, kind="ExternalInput")
    out = nc.dram_tensor("out", (S, D), MMD, kind="ExternalOutput")

    from contextlib import ExitStack

    with tile.TileContext(nc) as tc, ExitStack() as ctx:
        persist = ctx.enter_context(tc.tile_pool(name="persist", bufs=1))
        KT_sb = persist.tile([128, NP, S], MMD)        # K^T: pair p rows
        QT_sb = persist.tile([128, NP, S], MMD)        # Q^T
        V_sb = persist.tile([128, KT, HG, VW], MMD)    # V token-major + ones
        AON = persist.tile([128, NP, S], MMD)          # normalized AO^T
        wk_sb = persist.tile([128, CT, DHG], MMD)
        wq_sb = persist.tile([128, CT, DHG], MMD)
        wv_sb = persist.tile([128, CT, DHG], MMD)
        wo_sb = persist.tile([128, NP, D], MMD)
        bq_sb = persist.tile([128, NP], F32)
        bk_sb = persist.tile([128, NP], F32)
        bv_bc = persist.tile([128, DHG], F32)

        # wk first so the first K-projection matmuls start early
        nc.sync.dma_start(wk_sb[:], wk[:])
        nc.vector.memset(V_sb[:, :, :, DK], 1.0)       # ones columns
        # HAM warm-up: ~20 junk matmuls keep the PE busy from ~t=1us so
        # the clock gate opens (1.2 -> 2.4 GHz) before the first real
        # projection matmul, and the input-DMA wait isn't wasted
        wdum = persist.tile([128, QS], MMD)
        nc.vector.memset(wdum[:], 0.0)
        ones1 = persist.tile([33, DK], F32)            # tail rb broadcast
        nc.vector.memset(ones1[:], 1.0)
        ones1 = persist.tile([33, DK], F32)            # tail rb broadcast
        nc.vector.memset(ones1[:], 1.0)

        pmm = ctx.enter_context(tc.tile_pool(name="pmm", bufs=1, space="PSUM"))
        xpool = ctx.enter_context(tc.tile_pool(name="xw", bufs=7))
        vxpool = ctx.enter_context(tc.tile_pool(name="vxw", bufs=2))
        ptp = ctx.enter_context(tc.tile_pool(name="ptile", bufs=5))
        npool = ctx.enter_context(tc.tile_pool(name="norm", bufs=3))
        opool = ctx.enter_context(tc.tile_pool(name="ostage", bufs=3))
        dpool = ctx.enter_context(
            tc.tile_pool(name="dscratch", bufs=3, space="DRAM")
        )

        def window(xdram, qs, pool, dt, q=None):
            """One contiguous DMA for a 512-token slice (all 8 chunks)."""
            xc = pool.tile([128, CT, QS], dt, tag="xw", name="xw")
            (q or nc.sync).dma_start(
                xc[:], xdram[qs].rearrange("p (c n) -> p c n", n=QS)
            )
            return xc

        def kq_jt(win, w_sb, dst, b_sb, jt, qs):
            """dst[:, jt, qs] = w[:, :, jt].T @ x^T[:, qs] + bias."""
            ps = pmm.tile([128, QS], F32, tag="pj", name="pj", bufs=2)
            for ct in range(CT):
                nc.tensor.matmul(
                    ps[:],
                    w_sb[:, ct, jt * 128:(jt + 1) * 128],
                    win[:, ct, :],
                    start=(ct == 0), stop=(ct == CT - 1),
                )
            nc.vector.tensor_scalar_add(
                dst[:, jt, qs * QS:(qs + 1) * QS], ps[:], b_sb[:, jt:jt + 1]
            )

        def v_tiles(qs):
            """V_sb tok-tiles for one 512-token slice (4 tiles)."""
            win = window(xv, qs, vxpool, MMD)
            for i in range(4):
                tt = qs * 4 + i
                ps = pmm.tile([128, DHG], F32, tag="pj", name="pjv", bufs=2)
                for ct in range(CT):
                    nc.tensor.matmul(
                        ps[:],
                        win[:, ct, i * 128:(i + 1) * 128],
                        wv_sb[:, ct, :],
                        start=(ct == 0), stop=(ct == CT - 1),
                    )
                nc.vector.tensor_add(
                    V_sb[:, tt, :, 0:DK],
                    ps[:].rearrange("p (h d) -> p h d", d=DK),
                    bv_bc[:].rearrange("p (h d) -> p h d", d=DK),
                )

        def attention(p, qsb):
            """One head-pair over one 512-wide q-slice.

            Scores run k-major in the PE's 64x128 row-tiled mode (two heads
            at base partitions 0/64 execute concurrently). Softmax weights
            land as P^T; a ones column in V accumulates the denominator in
            PSUM row 64 during the PV matmuls. ScalarE k-tile-pairs emit P
            as fp8e4 and contract two k-tiles per DoubleRow AV pass;
            VectorE pairs keep fp16 P and classic AV (V stays fp8 lhsT).
            """
            q0 = qsb * QS
            ss_ktps = SS_KTPS[qsb]
            ao = [
                pmm.tile([VW, QS], F32, tag=f"ao{h2}", name=f"ao{h2}")
                for h2 in range(2)
            ]

            def av(pt, ktp):
                for j in range(2):
                    kt = 2 * ktp + j
                    for h2 in range(2):
                        nc.tensor.matmul(
                            ao[h2][:],
                            V_sb[:, kt, 2 * p + h2, 0:VW],
                            pt[j][:, h2, :],
                            start=(kt == 0), stop=(kt == KT - 1),
                        )

            # software-pipelined emission: AV(ktp-1) is emitted AFTER
            # scores(ktp), so the in-order PE queue streams scores while
            # the exp engines produce P for the previous pair -- the exp
            # latency hides behind score matmuls instead of stalling AV.
            prev = None
            for ktp in range(KTP):
                st = [
                    pmm.tile([128, 2, QS], F32, tag="st", name=f"st{j}",
                             bufs=2)
                    for j in range(2)
                ]
                for j in range(2):
                    kt = 2 * ktp + j
                    for h2 in range(2):
                        lo, hi = h2 * DK, h2 * DK + DK
                        nc.tensor.matmul(
                            st[j][:, h2, :],
                            KT_sb[lo:hi, p, kt * 128:(kt + 1) * 128],
                            QT_sb[lo:hi, p, q0:q0 + QS],
                            start=True, stop=True,
                        )
                pt = [
                    ptp.tile([128, 2, QS], MMD, tag="pt", name=f"pt{j}")
                    for j in range(2)
                ]
                for j in range(2):
                    if j == 1 and ktp not in ss_ktps:
                        nc.vector.tensor_scalar(
                            pt[j][:].bitcast(U16),
                            st[j][:],
                            FE_A, FE_B,
                            mybir.AluOpType.mult, mybir.AluOpType.add,
                        )
                    else:
                        nc.scalar.activation(
                            pt[j][:], st[j][:],
                            mybir.ActivationFunctionType.Exp,
                            scale=0.125,
                        )
                if prev is not None:
                    av(*prev)
                prev = (pt, ktp)
            av(*prev)
            tail = qsb == NQS - 1 and p == NP - 1
            tail_rr, tail_aos = [None], []
            for h2 in range(2):
                # copy to SBUF promptly so PSUM frees fast
                aos = npool.tile([VW, QS], F32, tag="aos", name="aos")
                if h2 == 0:
                    nc.scalar.copy(aos[:], ao[h2][:])
                else:
                    nc.vector.tensor_copy(aos[:], ao[h2][:])
                if tail:
                    # kernel-tail short chain: both heads' denominator
                    # rows staged at partitions 0 and 32 of ONE tile so
                    # a single reciprocal covers both (DVE recip is
                    # free-dim serial per lane: two [1,512] recips
                    # would serialize at 3.3us each); partition 32 is
                    # row-group aligned so the K=1 PE outer products
                    # auto-derive a valid tile_position
                    if h2 == 0:
                        tail_rr[0] = npool.tile([33, QS], F32, tag="rr",
                                                name="rr")
                        nc.scalar.copy(tail_rr[0][0:1, :],
                                       ao[h2][DK:VW, :])
                        tail_aos.append(aos)
                        continue
                    nc.vector.tensor_copy(tail_rr[0][32:33, :],
                                          ao[h2][DK:VW, :])
                    nc.vector.reciprocal(tail_rr[0][:], tail_rr[0][:])
                    tail_aos.append(aos)
                    for hh in range(2):
                        rbp = pmm.tile([DK, QS], F32, tag=f"ao{hh}",
                                       name="rbp")
                        nc.tensor.matmul(
                            rbp[:], ones1[32 * hh:32 * hh + 1, :],
                            tail_rr[0][32 * hh:32 * hh + 1, :],
                            start=True, stop=True,
                        )
                        nc.vector.tensor_mul(
                            AON[hh * DK:(hh + 1) * DK, p, q0:q0 + QS],
                            tail_aos[hh][0:DK, :],
                            rbp[:],
                        )
                    continue
                # full-lane recip via [1,512] -> [128,4] DRAM reshape
                rcd = dpool.tile([1, QS], F32, tag="rcd", name="rcd")
                dn = dpool.tile([1, QS], F32, tag="dn", name="dn")
                nc.sync.dma_start(dn[:], aos[DK:VW, :])
                rc = npool.tile([128, 4], F32, tag="rc", name="rc")
                nc.sync.dma_start(
                    rc[:], dn[:].rearrange("x (p j) -> (x p) j", j=4)
                )
                nc.vector.reciprocal(rc[:], rc[:])
                nc.sync.dma_start(
                    rcd[:].rearrange("x (p j) -> (x p) j", j=4), rc[:]
                )
                rb = npool.tile([DK, QS], F32, tag="rb", name="rb")
                nc.sync.dma_start(rb[:], bcast_ap(rcd[:], DK, QS))
                # gpsimd is idle so it takes the normalize muls
                nc.gpsimd.tensor_mul(
                    AON[h2 * DK:(h2 + 1) * DK, p, q0:q0 + QS],
                    aos[0:DK, :],
                    rb[:],
                )

        def outproj_tile(qsb, tt, last=False):
            """Out-projection for token tile tt (128 rows) of q-slice qsb."""
            q0 = qsb * QS
            ot = opool.tile([128, D], MMD, tag="ot", name="ot")
            for oh in range(2):
                po = pmm.tile([128, 512], F32, tag="pj", name="po", bufs=2)
                for ci in range(NP):
                    nc.tensor.matmul(
                        po[:],
                        AON[:, ci, q0 + tt * 128:q0 + (tt + 1) * 128],
                        wo_sb[:, ci, oh * 512:(oh + 1) * 512],
                        start=(ci == 0), stop=(ci == NP - 1),
                    )
                # VectorE carries the fast-exp + chores; PSUM evacuation
                # goes to ScalarE which has slack -- except at the very
                # end, where both engines split the two halves
                if last and oh == 0:
                    nc.vector.tensor_copy(
                        ot[:, oh * 512:(oh + 1) * 512], po[:])
                else:
                    nc.scalar.copy(ot[:, oh * 512:(oh + 1) * 512], po[:])
            # out-writes ride the Activation HWDGE queue: the Sync queue
            # carries windows + normalize chains and backs up
            nc.scalar.dma_start(
                out[q0 + tt * 128:q0 + (tt + 1) * 128, :], ot[:])

        # ---- emission schedule ---------------------------------------------
        # Normal priority: K proj (all pairs, windows shared across pairs),
        # Q pair-0 slice-0, V, then the attention stream + out-projections.
        # Remaining Q projections are demoted to background priority: the
        # scheduler pulls them early only when a data dependency demands it,
        # and otherwise uses them to fill PE idle slots.
        for w in range(72):
            pw = pmm.tile([128, QS], F32, tag="pj", name="pjw", bufs=2)
            nc.tensor.matmul(pw[:], wdum[:, 0:128], wdum[:],
                             start=True, stop=True)
        kwins = [window(xk, qs, xpool, MMD,
                        q=(nc.scalar if qs >= 2 else nc.sync))
                 for qs in range(NQS)]
        qwin0 = window(xq, 0, xpool, MMD)
        nc.sync.dma_start(bq_sb[:], bq[:])
        nc.sync.dma_start(bk_sb[:], bk[:])
        # weight loads ride the Activation HWDGE queue so they don't
        # queue behind the 1MB x-window transfers on the Sync queue
        nc.scalar.dma_start(wq_sb[:], wq[:])
        nc.scalar.dma_start(wv_sb[:], wv[:])
        nc.scalar.dma_start(bv_bc[:], bcast_ap(bv[:], 128, DHG))
        nc.scalar.dma_start(wo_sb[:], wo[:])
        for qs in range(NQS):
            kq_jt(kwins[qs], wk_sb, KT_sb, bk_sb, 0, qs)
        kq_jt(qwin0, wq_sb, QT_sb, bq_sb, 0, 0)
        for qs in range(NQS):
            v_tiles(qs)
        for jt in range(1, NP):
            for qs in range(NQS):
                kq_jt(kwins[qs], wk_sb, KT_sb, bk_sb, jt, qs)

        with tc.high_priority(offset=-(10 ** 6)):
            for jt in range(1, NP):
                kq_jt(qwin0, wq_sb, QT_sb, bq_sb, jt, 0)
            for qs in range(1, NQS):
                qwin = window(xq, qs, xpool, MMD)
                for jt in range(NP):
                    kq_jt(qwin, wq_sb, QT_sb, bq_sb, jt, qs)

        # two of each slice's out-projection tiles are deferred to the very
        # end: they fill the PE while the last softmax-normalize chain runs
        # (and keep the HAM clock warm for the final out-projections)
        for qsb in range(NQS):
            for p in range(NP):
                attention(p, qsb)
                if qsb > 0 and (p < 1 or qsb < NQS - 1):
                    outproj_tile(qsb - 1, p)
        for tt in range(1, NQS):
            outproj_tile(NQS - 2, tt)
        for tt in range(NQS):
            outproj_tile(NQS - 1, tt, last=(tt == NQS - 1))

    split_multi_waits(nc)
    return nc


def _prep_inputs(query, key, value, Wq, bq, Wk, bk, Wv, bv, Wo, bo):
    """Build the 8 per-core input maps (DMA-native DRAM layouts)."""
    F8NP = ml_dtypes.float8_e4m3

    def c16(a):
        return np.ascontiguousarray(a.astype(np.float16))

    def c8(a):
        return np.ascontiguousarray(
            np.clip(a, -240.0, 240.0).astype(F8NP))

    def xprep(x, b, dt):
        # x[b].T is (D, S) = (c*128, qs*512 tokens) feature-major ->
        # [qs, p, c*512] so a window DMA is contiguous per partition
        a = x[b].T.reshape(CT, 128, NQS, QS).transpose(2, 1, 0, 3)
        a = a.reshape(NQS, 128, CT * QS)
        return c16(a) if dt == "f16" else c8(a)

    def wprep(Wt, dt):
        # W[rows,:].T is (D, DHG) = (c*128, n) -> [p, c, n]
        a = Wt.reshape(CT, 128, DHG).transpose(1, 0, 2)
        return c16(a) if dt == "f16" else c8(a)

    in_maps = []
    for c in range(8):
        b, g = divmod(c, 2)
        rows = slice(g * DHG, (g + 1) * DHG)
        wo_a = Wo[:, rows].T.reshape(NP, 128, D).transpose(1, 0, 2)
        in_maps.append({
            "xq": xprep(query, b, "f16"),
            "xk": xprep(key, b, "f16"),
            "xv": xprep(value, b, "f16"),
            "wq": wprep(Wq[rows, :].T, "f16"),
            "wk": wprep(Wk[rows, :].T, "f16"),
            "wv": wprep(Wv[rows, :].T, "f16"),
            "wo": c16(wo_a),
            "bq": np.ascontiguousarray(bq[rows].reshape(NP, 128).T),
            "bk": np.ascontiguousarray(bk[rows].reshape(NP, 128).T),
            "bv": np.ascontiguousarray(bv[rows]),
        })
    return in_maps


_NC_CACHE = None


def run(inputs, trace=False):
    """Returns (full_output, BassKernelResults)."""
    global _NC_CACHE
    from concourse.bass_utils import run_bass_kernel_spmd

    inputs = {k: np.asarray(v, np.float32) for k, v in inputs.items()}
    in_maps = _prep_inputs(**inputs)
    if _NC_CACHE is None:
        _NC_CACHE = build_kernel()
    res = run_bass_kernel_spmd(
        _NC_CACHE, in_maps, core_ids=list(range(8)), trace=trace
    )
    bo = inputs["bo"]
    full = np.empty((B, S, D), np.float32)
    for b in range(B):
        full[b] = (res.results[2 * b]["out"].astype(np.float32)
                   + res.results[2 * b + 1]["out"].astype(np.float32) + bo)
    return full, res


def kernel(**inputs):
    return run(inputs, trace=False)[0]


# revision 54
# speedup vs baseline: 1.0252x; 1.0225x over previous
"""Multi-head attention (B=4, S=2048, D=1024, H=16) on 8 Trainium2 cores.

Sharding: (batch, head-group) grid — core c handles batch c//2, heads
(c%2)*8..(c%2)*8+8. Zero duplicated FLOPs; host sums the two partial
out-projections per batch and adds bo.

v9 design, ~397us (from trace analysis of the 443us v2: PE busy 358us/80%,
Scalar 202us exp, DVE 168us, 90us PE idle in startup/stalls/tail):
  * fp8 was tried (v3/v4) and is numerically DEAD for the 2e-2 gate:
    high-variance score rows concentrate softmax mass (k_eff ~ 40), so
    the ~4-7% fp8 quantization of P (or of q/k/v via fp8 projections)
    lands at 3-6e-2 output error (verified on HW and in numpy).
    Everything stays fp16; the late phase is PE-bound at ~95% occupancy.
  * Software-pipelined attention emission: AV(ktp-1) is emitted after
    scores(ktp), so the in-order PE queue streams scores while the exp
    engines produce P for the previous pair (v5 measured ~1us PE stalls
    per pair without this).
  * Per k-tile-pair, j=0's exp runs on ScalarE (exact ACTIVATE) and
    j=1's on VectorE (fp16 Schraudolph via uint16 convert; negatives
    saturate to +0.0), halving the exp latency per pair; SS_KTPS shifts
    two late-phase pairs fully onto ScalarE to balance VectorE's chores.
  * ~72 junk warm-up matmuls at t=0 bridge the input-DMA wait (~29us) and open
    the HAM clock gate (1.2 -> 2.4 GHz) before the first real matmul.
  * Host DRAM layouts are DMA-native (8KB descriptors: weights [p,c,n],
    x windows [qs,p,c*n]); weight loads, out-writes, and alternate startup
    K-windows ride the
    Activation HWDGE queue in parallel with x-windows/normalize chains
    on the Sync queue.
  * Output staged and written as fp16 (halves the 8.4MB/core out write;
    the host sums the two partial projections in fp32).
  * Kernel tail: three deferred out-projection tiles fill the last
    softmax-normalize gap; the final reciprocal broadcast skips the DMA
    round trip via a K=1 PE outer product (ones x recip-row).
  * Tail denominators staged at partitions 0 and 32 of one tile so a
    SINGLE reciprocal covers both heads (DVE recip is free-dim-serial
    per lane, 3.3us per call; partition 32 is row-group aligned so the
    K=1 outer products auto-derive a valid tile_position); the recip
    output and ones are fp16 so each outer product is a single-pass
    213ns matmul instead of fp32's two passes at 4cy/row. -3.8us mean
    total for the tail chain (385.6 then 383.9us mean measured).
    NOTE for the next session: with the tail chain now ~4us shorter,
    the three deferred out-projection tiles may overshoot the gap --
    re-tune the deferral count (2 vs 3) against a fresh trace.
  * Tried and REGRESSED (do not revisit blindly): splitting AON into
    per-slice tensors to break a false dep (433us -- scheduler side
    effects), batching the two per-unit reciprocal chains (467us), and
    a 44-warmup + xpool=6 combo (467us). The tile scheduler is very
    sensitive to emission/dependency perturbations; change one knob at
    a time and re-measure.
"""

import numpy as np
import ml_dtypes

import bass_rust
import concourse.bass as bass
import concourse.tile as tile
from concourse import mybir

F32 = mybir.dt.float32
U16 = mybir.dt.uint16
MMD = mybir.dt.float16     # fp16 matmul operand dtype

B, S, D = 4, 2048, 1024
NH, DK = 16, 64            # total heads, head dim
HG = 8                     # heads per core (head group)
DHG = HG * DK              # 512 features per head group
NP = 4                     # pairs of heads per core
QS = 512                   # q-slice size
NQS = S // QS              # 4
KT = S // 128              # 16 k-tiles
KTP = KT // 2              # 8 k-tile pairs
CT = D // 128              # 8 contraction chunks for projections
VW = DK + 1                # 65: V columns per head incl. ones column

# fp16 Schraudolph fast-exp: round(s*1024*0.125/ln2 + 15360-57.77) as
# uint16 IS the fp16 bit pattern of exp(s/8) to within +-3.5% (zero mean)
FE_A = 1024.0 * 0.125 / float(np.log(2.0))
FE_B = 15360.0 - 57.77

# k-tile-pairs where ScalarE takes BOTH exp tiles (instead of the default
# j0->Scalar, j1->Vector split): late q-slices shift work off VectorE,
# which also carries the reciprocal/copy chores.
SS_KTPS = {0: (), 1: (), 2: (2, 5), 3: (2, 5)}


def split_multi_waits(nc):
    """This toolchain's walrus accepts only ONE sync-wait per instruction;
    Tile attaches several (one per producer proc). Hoist all but one wait
    onto single-wait NOPs inserted just before the instruction on the same
    engine (engines are in-order, so semantics are identical)."""
    uid = 0
    for f in nc.m.functions:
        for bb in f.blocks:
            il = bb.instructions
            i = 0
            while i < len(il):
                inst = il[i]
                si = inst.sync_info
                if si is not None and len(si.on_wait) > 1:
                    waits = list(si.on_wait)
                    inst.sync_info = bass_rust.SyncInfo(
                        on_wait=[waits[-1]], on_update=list(si.on_update)
                    )
                    for w in waits[:-1]:
                        nop = mybir.InstNoOp(
                            name=f"WSPLIT-{uid}",
                            engine=inst.engine,
                            bass_nofuse=True,
                            sync_info=bass_rust.SyncInfo(
                                on_wait=[w], on_update=[]
                            ),
                        )
                        uid += 1
                        il.insert(i, nop)
                        i += 1
                i += 1


def bcast_ap(ap, parts, n):
    """Partition-broadcast view of a DRAM row AP: [[0,parts],[1,n]]."""
    return bass.AP(tensor=ap.tensor, offset=ap.offset, ap=[[0, parts], [1, n]])


def build_kernel():
    nc = bass.Bass(trn_type="TRN2")

    # host layouts are already DMA-native (see _prep_inputs)
    xq = nc.dram_tensor("xq", (NQS, 128, CT * QS), MMD, kind="ExternalInput")
    xk = nc.dram_tensor("xk", (NQS, 128, CT * QS), MMD, kind="ExternalInput")
    xv = nc.dram_tensor("xv", (NQS, 128, CT * QS), MMD, kind="ExternalInput")
    wq = nc.dram_tensor("wq", (128, CT, DHG), MMD, kind="ExternalInput")
    wk = nc.dram_tensor("wk", (128, CT, DHG), MMD, kind="ExternalInput")
    wv = nc.dram_tensor("wv", (128, CT, DHG), MMD, kind="ExternalInput")
    wo = nc.dram_tensor("wo", (128, NP, D), MMD, kind="ExternalInput")
    bq = nc.dram_tensor("bq", (128, NP), F32, kind="ExternalInput")
    bk = nc.dram_tensor("bk", (128, NP), F32, kind="ExternalInput")
    bv = nc.dram_tensor("bv", (DHG,), F32, kind="ExternalInput")
    out = nc.dram_tensor("out", (S, D), MMD, kind="ExternalOutput")

    from contextlib import ExitStack

    with tile.TileContext(nc) as tc, ExitStack() as ctx:
        persist = ctx.enter_context(tc.tile_pool(name="persist", bufs=1))
        KT_sb = persist.tile([128, NP, S], MMD)        # K^T: pair p rows
        QT_sb = persist.tile([128, NP, S], MMD)        # Q^T
        V_sb = persist.tile([128, KT, HG, VW], MMD)    # V token-major + ones
        AON = persist.tile([128, NP, S], MMD)          # normalized AO^T
        wk_sb = persist.tile([128, CT, DHG], MMD)
        wq_sb = persist.tile([128, CT, DHG], MMD)
        wv_sb = persist.tile([128, CT, DHG], MMD)
        wo_sb = persist.tile([128, NP, D], MMD)
        bq_sb = persist.tile([128, NP], F32)
        bk_sb = persist.tile([128, NP], F32)
        bv_bc = persist.tile([128, DHG], F32)

        # wk first so the first K-projection matmuls start early
        nc.sync.dma_start(wk_sb[:], wk[:])
        nc.vector.memset(V_sb[:, :, :, DK], 1.0)       # ones columns
        # HAM warm-up: ~20 junk matmuls keep the PE busy from ~t=1us so
        # the clock gate opens (1.2 -> 2.4 GHz) before the first real
        # projection matmul, and the input-DMA wait isn't wasted
        wdum = persist.tile([128, QS], MMD)
        nc.vector.memset(wdum[:], 0.0)
        ones1 = persist.tile([33, DK], MMD)            # tail rb broadcast
        nc.vector.memset(ones1[:], 1.0)
        ones1 = persist.tile([33, DK], MMD)            # tail rb broadcast
        nc.vector.memset(ones1[:], 1.0)

        pmm = ctx.enter_context(tc.tile_pool(name="pmm", bufs=1, space="PSUM"))
        xpool = ctx.enter_context(tc.tile_pool(name="xw", bufs=7))
        vxpool = ctx.enter_context(tc.tile_pool(name="vxw", bufs=2))
        ptp = ctx.enter_context(tc.tile_pool(name="ptile", bufs=5))
        npool = ctx.enter_context(tc.tile_pool(name="norm", bufs=3))
        opool = ctx.enter_context(tc.tile_pool(name="ostage", bufs=3))
        dpool = ctx.enter_context(
            tc.tile_pool(name="dscratch", bufs=3, space="DRAM")
        )

        def window(xdram, qs, pool, dt, q=None):
            """One contiguous DMA for a 512-token slice (all 8 chunks)."""
            xc = pool.tile([128, CT, QS], dt, tag="xw", name="xw")
            (q or nc.sync).dma_start(
                xc[:], xdram[qs].rearrange("p (c n) -> p c n", n=QS)
            )
            return xc

        def kq_jt(win, w_sb, dst, b_sb, jt, qs):
            """dst[:, jt, qs] = w[:, :, jt].T @ x^T[:, qs] + bias."""
            ps = pmm.tile([128, QS], F32, tag="pj", name="pj", bufs=2)
            for ct in range(CT):
                nc.tensor.matmul(
                    ps[:],
                    w_sb[:, ct, jt * 128:(jt + 1) * 128],
                    win[:, ct, :],
                    start=(ct == 0), stop=(ct == CT - 1),
                )
            nc.vector.tensor_scalar_add(
                dst[:, jt, qs * QS:(qs + 1) * QS], ps[:], b_sb[:, jt:jt + 1]
            )

        def v_tiles(qs):
            """V_sb tok-tiles for one 512-token slice (4 tiles)."""
            win = window(xv, qs, vxpool, MMD)
            for i in range(4):
                tt = qs * 4 + i
                ps = pmm.tile([128, DHG], F32, tag="pj", name="pjv", bufs=2)
                for ct in range(CT):
                    nc.tensor.matmul(
                        ps[:],
                        win[:, ct, i * 128:(i + 1) * 128],
                        wv_sb[:, ct, :],
                        start=(ct == 0), stop=(ct == CT - 1),
                    )
                nc.vector.tensor_add(
                    V_sb[:, tt, :, 0:DK],
                    ps[:].rearrange("p (h d) -> p h d", d=DK),
                    bv_bc[:].rearrange("p (h d) -> p h d", d=DK),
                )

        def attention(p, qsb):
            """One head-pair over one 512-wide q-slice.

            Scores run k-major in the PE's 64x128 row-tiled mode (two heads
            at base partitions 0/64 execute concurrently). Softmax weights
            land as P^T; a ones column in V accumulates the denominator in
            PSUM row 64 during the PV matmuls. ScalarE k-tile-pairs emit P
            as fp8e4 and contract two k-tiles per DoubleRow AV pass;
            VectorE pairs keep fp16 P and classic AV (V stays fp8 lhsT).
            """
            q0 = qsb * QS
            ss_ktps = SS_KTPS[qsb]
            ao = [
                pmm.tile([VW, QS], F32, tag=f"ao{h2}", name=f"ao{h2}")
                for h2 in range(2)
            ]

            def av(pt, ktp):
                for j in range(2):
                    kt = 2 * ktp + j
                    for h2 in range(2):
                        nc.tensor.matmul(
                            ao[h2][:],
                            V_sb[:, kt, 2 * p + h2, 0:VW],
                            pt[j][:, h2, :],
                            start=(kt == 0), stop=(kt == KT - 1),
                        )

            # software-pipelined emission: AV(ktp-1) is emitted AFTER
            # scores(ktp), so the in-order PE queue streams scores while
            # the exp engines produce P for the previous pair -- the exp
            # latency hides behind score matmuls instead of stalling AV.
            prev = None
            for ktp in range(KTP):
                st = [
                    pmm.tile([128, 2, QS], F32, tag="st", name=f"st{j}",
                             bufs=2)
                    for j in range(2)
                ]
                for j in range(2):
                    kt = 2 * ktp + j
                    for h2 in range(2):
                        lo, hi = h2 * DK, h2 * DK + DK
                        nc.tensor.matmul(
                            st[j][:, h2, :],
                            KT_sb[lo:hi, p, kt * 128:(kt + 1) * 128],
                            QT_sb[lo:hi, p, q0:q0 + QS],
                            start=True, stop=True,
                        )
                pt = [
                    ptp.tile([128, 2, QS], MMD, tag="pt", name=f"pt{j}")
                    for j in range(2)
                ]
                for j in range(2):
                    if j == 1 and ktp not in ss_ktps:
                        nc.vector.tensor_scalar(
                            pt[j][:].bitcast(U16),
                            st[j][:],
                            FE_A, FE_B,
                            mybir.AluOpType.mult, mybir.AluOpType.add,
                        )
                    else:
                        nc.scalar.activation(
                            pt[j][:], st[j][:],
                            mybir.ActivationFunctionType.Exp,
                            scale=0.125,
                        )
                if prev is not None:
                    av(*prev)
                prev = (pt, ktp)
            av(*prev)
            tail = qsb == NQS - 1 and p == NP - 1
            tail_rr, tail_aos = [None], []
            for h2 in range(2):
                # copy to SBUF promptly so PSUM frees fast
                aos = npool.tile([VW, QS], F32, tag="aos", name="aos")
                if h2 == 0:
                    nc.scalar.copy(aos[:], ao[h2][:])
                else:
                    nc.vector.tensor_copy(aos[:], ao[h2][:])
                if tail:
                    # kernel-tail short chain: both heads' denominator
                    # rows staged at partitions 0 and 32 of ONE tile so
                    # a single reciprocal covers both (DVE recip is
                    # free-dim serial per lane: two [1,512] recips
                    # would serialize at 3.3us each); partition 32 is
                    # row-group aligned so the K=1 PE outer products
                    # auto-derive a valid tile_position
                    if h2 == 0:
                        tail_rr[0] = npool.tile([33, QS], F32, tag="rr",
                                                name="rr")
                        tail_rr.append(npool.tile([33, QS], MMD,
                                                  tag="rh", name="rh",
                                                  bufs=1))
                        nc.scalar.copy(tail_rr[0][0:1, :],
                                       ao[h2][DK:VW, :])
                        tail_aos.append(aos)
                        continue
                    nc.vector.tensor_copy(tail_rr[0][32:33, :],
                                          ao[h2][DK:VW, :])
                    # fp16 reciprocal output: recip values ~3e-4 are far
                    # inside fp16 range; enables single-pass fp16 outer
                    # products (fp32 matmul needs 2 passes at 4cy/row)
                    with nc.allow_low_precision(reason="tail recip fp16"):
                        nc.vector.reciprocal(tail_rr[1][:], tail_rr[0][:])
                    tail_aos.append(aos)
                    for hh in range(2):
                        rbp = pmm.tile([DK, QS], F32, tag=f"ao{hh}",
                                       name="rbp")
                        nc.tensor.matmul(
                            rbp[:], ones1[32 * hh:32 * hh + 1, :],
                            tail_rr[1][32 * hh:32 * hh + 1, :],
                            start=True, stop=True,
                        )
                        nc.vector.tensor_mul(
                            AON[hh * DK:(hh + 1) * DK, p, q0:q0 + QS],
                            tail_aos[hh][0:DK, :],
                            rbp[:],
                        )
                    continue
                # full-lane recip via [1,512] -> [128,4] DRAM reshape
                rcd = dpool.tile([1, QS], F32, tag="rcd", name="rcd")
                dn = dpool.tile([1, QS], F32, tag="dn", name="dn")
                nc.sync.dma_start(dn[:], aos[DK:VW, :])
                rc = npool.tile([128, 4], F32, tag="rc", name="rc")
                nc.sync.dma_start(
                    rc[:], dn[:].rearrange("x (p j) -> (x p) j", j=4)
                )
                nc.vector.reciprocal(rc[:], rc[:])
                nc.sync.dma_start(
                    rcd[:].rearrange("x (p j) -> (x p) j", j=4), rc[:]
                )
                rb = npool.tile([DK, QS], F32, tag="rb", name="rb")
                nc.sync.dma_start(rb[:], bcast_ap(rcd[:], DK, QS))
                # gpsimd is idle so it takes the normalize muls
                nc.gpsimd.tensor_mul(
                    AON[h2 * DK:(h2 + 1) * DK, p, q0:q0 + QS],
                    aos[0:DK, :],
                    rb[:],
                )

        def outproj_tile(qsb, tt, last=False):
            """Out-projection for token tile tt (128 rows) of q-slice qsb."""
            q0 = qsb * QS
            ot = opool.tile([128, D], MMD, tag="ot", name="ot")
            for oh in range(2):
                po = pmm.tile([128, 512], F32, tag="pj", name="po", bufs=2)
                for ci in range(NP):
                    nc.tensor.matmul(
                        po[:],
                        AON[:, ci, q0 + tt * 128:q0 + (tt + 1) * 128],
                        wo_sb[:, ci, oh * 512:(oh + 1) * 512],
                        start=(ci == 0), stop=(ci == NP - 1),
                    )
                # VectorE carries the fast-exp + chores; PSUM evacuation
                # goes to ScalarE which has slack -- except at the very
                # end, where both engines split the two halves
                if last and oh == 0:
                    nc.vector.tensor_copy(
                        ot[:, oh * 512:(oh + 1) * 512], po[:])
                else:
                    nc.scalar.copy(ot[:, oh * 512:(oh + 1) * 512], po[:])
            # out-writes ride the Activation HWDGE queue: the Sync queue
            # carries windows + normalize chains and backs up
            nc.scalar.dma_start(
                out[q0 + tt * 128:q0 + (tt + 1) * 128, :], ot[:])

        # ---- emission schedule ---------------------------------------------
        # Normal priority: K proj (all pairs, windows shared across pairs),
        # Q pair-0 slice-0, V, then the attention stream + out-projections.
        # Remaining Q projections are demoted to background priority: the
        # scheduler pulls them early only when a data dependency demands it,
        # and otherwise uses them to fill PE idle slots.
        for w in range(72):
            pw = pmm.tile([128, QS], F32, tag="pj", name="pjw", bufs=2)
            nc.tensor.matmul(pw[:], wdum[:, 0:128], wdum[:],
                             start=True, stop=True)
        kwins = [window(xk, qs, xpool, MMD,
                        q=(nc.scalar if qs >= 2 else nc.sync))
                 for qs in range(NQS)]
        qwin0 = window(xq, 0, xpool, MMD)
        nc.sync.dma_start(bq_sb[:], bq[:])
        nc.sync.dma_start(bk_sb[:], bk[:])
        # weight loads ride the Activation HWDGE queue so they don't
        # queue behind the 1MB x-window transfers on the Sync queue
        nc.scalar.dma_start(wq_sb[:], wq[:])
        nc.scalar.dma_start(wv_sb[:], wv[:])
        nc.scalar.dma_start(bv_bc[:], bcast_ap(bv[:], 128, DHG))
        nc.scalar.dma_start(wo_sb[:], wo[:])
        for qs in range(NQS):
            kq_jt(kwins[qs], wk_sb, KT_sb, bk_sb, 0, qs)
        kq_jt(qwin0, wq_sb, QT_sb, bq_sb, 0, 0)
        for qs in range(NQS):
            v_tiles(qs)
        for jt in range(1, NP):
            for qs in range(NQS):
                kq_jt(kwins[qs], wk_sb, KT_sb, bk_sb, jt, qs)

        with tc.high_priority(offset=-(10 ** 6)):
            for jt in range(1, NP):
                kq_jt(qwin0, wq_sb, QT_sb, bq_sb, jt, 0)
            for qs in range(1, NQS):
                qwin = window(xq, qs, xpool, MMD)
                for jt in range(NP):
                    kq_jt(qwin, wq_sb, QT_sb, bq_sb, jt, qs)

        # two of each slice's out-projection tiles are deferred to the very
        # end: they fill the PE while the last softmax-normalize chain runs
        # (and keep the HAM clock warm for the final out-projections)
        for qsb in range(NQS):
            for p in range(NP):
                attention(p, qsb)
                if qsb > 0 and (p < 1 or qsb < NQS - 1):
                    outproj_tile(qsb - 1, p)
        for tt in range(1, NQS):
            outproj_tile(NQS - 2, tt)
        for tt in range(NQS):
            outproj_tile(NQS - 1, tt, last=(tt == NQS - 1))

    split_multi_waits(nc)
    return nc


def _prep_inputs(query, key, value, Wq, bq, Wk, bk, Wv, bv, Wo, bo):
    """Build the 8 per-core input maps (DMA-native DRAM layouts)."""
    F8NP = ml_dtypes.float8_e4m3

    def c16(a):
        return np.ascontiguousarray(a.astype(np.float16))

    def c8(a):
        return np.ascontiguousarray(
            np.clip(a, -240.0, 240.0).astype(F8NP))

    def xprep(x, b, dt):
        # x[b].T is (D, S) = (c*128, qs*512 tokens) feature-major ->
        # [qs, p, c*512] so a window DMA is contiguous per partition
        a = x[b].T.reshape(CT, 128, NQS, QS).transpose(2, 1, 0, 3)
        a = a.reshape(NQS, 128, CT * QS)
        return c16(a) if dt == "f16" else c8(a)

    def wprep(Wt, dt):
        # W[rows,:].T is (D, DHG) = (c*128, n) -> [p, c, n]
        a = Wt.reshape(CT, 128, DHG).transpose(1, 0, 2)
        return c16(a) if dt == "f16" else c8(a)

    in_maps = []
    for c in range(8):
        b, g = divmod(c, 2)
        rows = slice(g * DHG, (g + 1) * DHG)
        wo_a = Wo[:, rows].T.reshape(NP, 128, D).transpose(1, 0, 2)
        in_maps.append({
            "xq": xprep(query, b, "f16"),
            "xk": xprep(key, b, "f16"),
            "xv": xprep(value, b, "f16"),
            "wq": wprep(Wq[rows, :].T, "f16"),
            "wk": wprep(Wk[rows, :].T, "f16"),
            "wv": wprep(Wv[rows, :].T, "f16"),
            "wo": c16(wo_a),
            "bq": np.ascontiguousarray(bq[rows].reshape(NP, 128).T),
            "bk": np.ascontiguousarray(bk[rows].reshape(NP, 128).T),
            "bv": np.ascontiguousarray(bv[rows]),
        })
    return in_maps


_NC_CACHE = None


def run(inputs, trace=False):
    """Returns (full_output, BassKernelResults)."""
    global _NC_CACHE
    from concourse.bass_utils import run_bass_kernel_spmd

    inputs = {k: np.asarray(v, np.float32) for k, v in inputs.items()}
    in_maps = _prep_inputs(**inputs)
    if _NC_CACHE is None:
        _NC_CACHE = build_kernel()
    res = run_bass_kernel_spmd(
        _NC_CACHE, in_maps, core_ids=list(range(8)), trace=trace
    )
    bo = inputs["bo"]
    full = np.empty((B, S, D), np.float32)
    for b in range(B):
        full[b] = (res.results[2 * b]["out"].astype(np.float32)
                   + res.results[2 * b + 1]["out"].astype(np.float32) + bo)
    return full, res


def kernel(**inputs):
    return run(inputs, trace=False)[0]
